# revision 23
# baseline (speedup 1.0000x reference)
"""Masked multi-head attention kernel for Trainium2 (Bass/Tile), 8-core SPMD.

v9.1 — engine-rebalanced on top of v8's load paths:
  - QK matmuls for the two heads of a pair are interleaved at chunk level so
    their K=64 row-tiles (partitions 0-63 / 64-127) run CONCURRENTLY in the
    PE array (~1.8x measured).
  - The softmax work is split across engines:
      exp:  Act on 13/16 of key-chunks, DVE Schraudolph bits-trick
            ((st*A+B) -> int16 -> bitcast bf16) on 3/16.
      mask: DVE bf16 tensor_mul (2x mode) on 11/16, gpsimd on 2/16, and on
            3/16 a PE identity-matmul accumulates -448*mask into the scores
            PSUM pre-exp (those slab rows are host-complemented).
  - AV matmuls are emitted two groups late (software pipeline) so the PE
    never waits on the exp/mask chain.
  - The next pair's Q/K/V/mask DMAs are emitted during the current pair's
    qc==2 so SWDGE descriptor generation and transfers overlap compute
    (this was ~25us of idle at every pair boundary). qc3's pool-mask muls
    run on DVE so they don't queue behind the prefetch DIRECT2Ds.
  - po accumulates all 16 key-chunks in one PSUM group; the host divides by
    the denominator row and transposes, as in v8.
"""

import os
import sys

sys.path.insert(0, "/opt/trn_rl_repo")

import numpy as np

import concourse.bass as bass
import concourse.mybir as mybir
import concourse.tile as tile
from concourse import bacc
from concourse.bass_utils import run_bass_kernel_spmd
from concourse.masks import make_identity

N_CORES = 8
BH, S_FULL, D = 64, 2048, 64
H_PER_CORE = BH // N_CORES  # 8
P = 128
KCH = 128
QCH = 512
SCALE = 1.0 / 32.0
MASK_BIAS = -448.0  # exp(-448/32) = exp(-14) ~ 8e-7

# per-(head, qc) chunk assignments (chunk = 128 keys; 16 chunks of S=2048)
GROUPS = tuple((c, c + 2) for c in range(0, 16, 2))
M2_CHUNKS = frozenset({6, 7, 15})  # mask added into PSUM by PE identity matmul
POOL_CHUNKS = frozenset({4, 5, 13, 14})  # mask multiplied on gpsimd
SCHRAU_GROUPS = frozenset({5, 6})  # exp computed on DVE via Schraudolph bits
SCHRAU_A = (128.0 / float(np.log(2.0))) * SCALE  # 5.7708
SCHRAU_B = 127.0 * 128.0 - 8.0

n_kch = S_FULL // KCH  # 16
n_qc = S_FULL // QCH  # 4
n_quart = 4
KLQ = n_kch // n_quart  # 4
QW = KLQ * QCH  # 2048


def build_attention(tc, o_ap, q_ap, k_ap, v_ap, m_ap, H, S):
    nc = tc.nc
    dt = mybir.dt
    n_pairs = H // 2
    GW = 2 * QCH

    with (
        tc.tile_pool(name="const", bufs=1) as constp,
        tc.tile_pool(name="qkslab", bufs=2) as qkp,
        tc.tile_pool(name="vst", bufs=4) as vp,
        tc.tile_pool(name="maskp", bufs=16) as maskp,
        tc.tile_pool(name="ptp", bufs=10) as ptp,
        tc.tile_pool(name="osbp", bufs=6) as osbp,
        tc.tile_pool(name="ps_s", bufs=3, space="PSUM") as ps_s,
        tc.tile_pool(name="ps_po", bufs=2, space="PSUM") as ps_po,
    ):
        # -448 * I128 stationary for the PE mask-add matmuls
        negi = constp.tile([P, P], dt.bfloat16)
        make_identity(nc, negi[:])
        nc.vector.tensor_scalar_mul(negi[:], negi[:], MASK_BIAS)

        # PE warmup through the initial DMA wait (HAM -> K=8/8).
        wsrc = constp.tile([P, QCH], dt.bfloat16)
        nc.vector.memset(wsrc[:], 0.0)
        wps = ps_s.tile([P, GW], dt.float32, tag="st")
        for _ in range(12):
            nc.tensor.matmul(
                wps[:, 0:QCH], wsrc[:, 0:P], wsrc[:], start=True, stop=True
            )

        def load_pair(pr):
            """Emit the DMA loads for pair pr; returns slab handles."""
            heads = (2 * pr, 2 * pr + 1)
            slabs = {}
            for name, src_ap in (("q", q_ap), ("k", k_ap)):
                slab = qkp.tile([P, S], dt.bfloat16, tag=f"{name}t2", name=name)
                for hi, h in enumerate(heads):
                    nc.sync.dma_start(slab[hi * D : (hi + 1) * D, :], src_ap[h])
                slabs[name] = slab
            vst = [None, None]
            for hi, h in enumerate(heads):
                vt = vp.tile([P, n_kch * (D + 1)], dt.bfloat16, tag="vst", name=f"v{hi}")
                vt3 = vt[:].rearrange("p (t c) -> p t c", c=D + 1)
                nc.sync.dma_start(
                    vt3[:, :, 0:D], v_ap[h].rearrange("(t p) d -> p t d", p=P)
                )
                nc.vector.memset(vt3[:, :, D : D + 1], 1.0)
                vst[hi] = vt
            return {"heads": heads, "QT2": slabs["q"], "KT2": slabs["k"],
                    "vst": vst, "mslabs": [[[None] * 2 for _ in range(n_quart)]
                                           for _ in range(2)]}

        def load_masks(state, qh):
            """Mask eighth-slabs (quarter x qc-half) for one pair: u8->bf16
            SWDGE cast from the host's pre-arranged layout (contiguous 4KB
            per partition row -> ~130 descriptors per DMA). value 1 = keep
            (multiply chunks) or masked (M2 chunks; host complements those
            key rows)."""
            heads = state["heads"]
            for qt in range(n_quart):
                for hi, h in enumerate(heads):
                    ms = maskp.tile(
                        [P, 2 * KLQ * QCH], dt.bfloat16, tag="ms",
                        name=f"ms{qt}_{hi}_{qh}",
                    )
                    nc.gpsimd.dma_start(ms[:], m_ap[h, qt, qh])
                    state["mslabs"][hi][qt][qh] = ms

        cur = load_pair(0)
        load_masks(cur, 0)
        load_masks(cur, 1)
        n_units = n_qc * len(GROUPS)  # 32 (qc, group) units per head stream
        LAG = 1  # hi1 stream lags hi0 by one slot
        AV_DELAY = 3  # AV emitted this many slots after its QK slot

        for pr in range(n_pairs):
            heads = cur["heads"]
            QT2, KT2, vst, mslabs = cur["QT2"], cur["KT2"], cur["vst"], cur["mslabs"]

            def ms_slice(hi, c, qc, span=1):
                qt, kl = c // KLQ, c % KLQ
                ms = mslabs[hi][qt][qc // 2]
                off = (qc % 2) * QW + kl * QCH
                return ms[:, off : off + span * QCH]

            nxt = None
            po = {}  # (hi, qc) -> psum tile
            pts = {}  # (hi, unit) -> pt tile

            def get_po(hi, qc):
                if (hi, qc) not in po:
                    po[(hi, qc)] = ps_po.tile(
                        [D + 1, QCH], dt.float32, tag="po", name=f"po{hi}_{qc}"
                    )
                return po[(hi, qc)]

            def emit_front(hi, unit, st):
                """QK chunks for (hi, unit) into st (emitted pre-interleaved
                by the caller), then m2 / exp / mask."""
                qc, gi = divmod(unit, len(GROUPS))
                c0, c1 = GROUPS[gi]
                nch = c1 - c0
                # PE mask-add: st += -448 * mask (full-array matmul)
                for c in range(c0, c1):
                    if c not in M2_CHUNKS:
                        continue
                    nc.tensor.matmul(
                        st[:, (c - c0) * QCH : (c - c0 + 1) * QCH],
                        negi[:],
                        ms_slice(hi, c, qc),
                        start=False,
                        stop=True,
                    )
                # exp
                pt = ptp.tile([P, nch * QCH], dt.bfloat16, tag="pt", name=f"pt{hi}")
                if gi in SCHRAU_GROUPS:
                    nc.vector.tensor_scalar(
                        pt[:].bitcast(dt.int16),
                        st[:],
                        SCHRAU_A,
                        SCHRAU_B,
                        mybir.AluOpType.mult,
                        mybir.AluOpType.add,
                    )
                else:
                    nc.scalar.activation(
                        pt[:],
                        st[:],
                        mybir.ActivationFunctionType.Exp,
                        scale=SCALE,
                    )
                # mask multiplies (skip M2 chunks; POOL chunks on gpsimd,
                # except during qc3 when gpsimd must run the prefetch)
                c = c0
                while c < c1:
                    if c in M2_CHUNKS:
                        c += 1
                        continue
                    span = 1
                    while (
                        c + span < c1
                        and (c + span) not in M2_CHUNKS
                        and ((c + span) in POOL_CHUNKS) == (c in POOL_CHUNKS)
                        and (c + span) // KLQ == c // KLQ
                    ):
                        span += 1
                    # pool masks colliding with the next pair's mask-DMA
                    # descriptor generation (qc2 early chunks, qc3 late
                    # chunks) go to DVE instead
                    use_pool = c in POOL_CHUNKS and not (
                        (qc == 2 and c < 8) or (qc == 3 and c >= 8)
                    )
                    eng = nc.gpsimd if use_pool else nc.vector
                    p0 = (c - c0) * QCH
                    eng.tensor_mul(
                        pt[:, p0 : p0 + span * QCH],
                        pt[:, p0 : p0 + span * QCH],
                        ms_slice(hi, c, qc, span),
                    )
                    c += span
                pts[(hi, unit)] = pt

            def emit_av(hi, unit):
                qc, gi = divmod(unit, len(GROUPS))
                c0, c1 = GROUPS[gi]
                pt = pts.pop((hi, unit))
                p = get_po(hi, qc)
                for c in range(c0, c1):
                    nc.tensor.matmul(
                        p[:],
                        vst[hi][:, c * (D + 1) : (c + 1) * (D + 1)],
                        pt[:, (c - c0) * QCH : (c - c0 + 1) * QCH],
                        start=(c == 0),
                        stop=(c == n_kch - 1),
                        skip_group_check=True,
                    )
                if gi == len(GROUPS) - 1:
                    # last group of this qc: drain po
                    h = heads[hi]
                    ot = osbp.tile([D + 1, QCH], dt.float32, tag="osb")
                    nc.vector.tensor_copy(ot[:], p[:])
                    nc.sync.dma_start(o_ap[h, qc], ot[:])
                    del po[(hi, qc)]

            for slot in range(n_units + LAG + AV_DELAY + 1):
                parts = []
                if slot < n_units:
                    parts.append((0, slot))
                if LAG <= slot < n_units + LAG:
                    parts.append((1, slot - LAG))
                # QK matmuls, chunk-interleaved across the two streams so
                # their K=64 row-tiles (rows 0-63 / 64-127) run concurrently
                sts = {}
                for hi, unit in parts:
                    qc, gi = divmod(unit, len(GROUPS))
                    c0, c1 = GROUPS[gi]
                    sts[hi] = ps_s.tile(
                        [P, (c1 - c0) * QCH], dt.float32, tag="st", name=f"st{hi}"
                    )
                for i in range(2):
                    for hi, unit in parts:
                        qc, gi = divmod(unit, len(GROUPS))
                        c0, c1 = GROUPS[gi]
                        if c0 + i >= c1:
                            continue
                        c = c0 + i
                        nc.tensor.matmul(
                            sts[hi][:, i * QCH : (i + 1) * QCH],
                            KT2[hi * D : (hi + 1) * D, c * KCH : (c + 1) * KCH],
                            QT2[
                                hi * D : (hi + 1) * D,
                                qc * QCH : (qc + 1) * QCH,
                            ],
                            start=True,
                            stop=c not in M2_CHUNKS,
                        )
                for hi, unit in parts:
                    emit_front(hi, unit, sts[hi])
                # delayed AV per stream
                u0 = slot - AV_DELAY
                if 0 <= u0 < n_units:
                    emit_av(0, u0)
                u1 = slot - LAG - AV_DELAY
                if 0 <= u1 < n_units:
                    emit_av(1, u1)
                # prefetch the next pair's slabs: mask halves as soon as the
                # ring frees them (qh0 after qc1, qh1 after qc3 starts), and
                # Q/K/V late in qc3
                if pr + 1 < n_pairs:
                    if slot == 2 * len(GROUPS) + 1:
                        nxt = load_pair(pr + 1)
                        load_masks(nxt, 0)
                    elif slot == 3 * len(GROUPS) + 3:
                        load_masks(nxt, 1)
            cur = nxt


def build_program(H=H_PER_CORE, S=S_FULL, **flags):
    nc = bacc.Bacc()
    q = nc.dram_tensor("q", [H, D, S], mybir.dt.bfloat16, kind="ExternalInput")
    k = nc.dram_tensor("k", [H, D, S], mybir.dt.bfloat16, kind="ExternalInput")
    v = nc.dram_tensor("v", [H, S, D], mybir.dt.bfloat16, kind="ExternalInput")
    # mask pre-arranged on host: [h, quarter, qc-half, partition, qc|kl|j]
    m = nc.dram_tensor(
        "m", [H, 4, 2, P, 2 * KLQ * QCH], mybir.dt.uint8, kind="ExternalInput"
    )
    # unnormalized O^T per q-chunk: [head, qc, d|l, q] — host normalizes
    o = nc.dram_tensor(
        "o", [H, S // QCH, D + 1, QCH], mybir.dt.float32, kind="ExternalOutput"
    )
    with tile.TileContext(nc) as tc:
        build_attention(tc, o.ap(), q.ap(), k.ap(), v.ap(), m.ap(), H=H, S=S, **flags)
    nc.compile()
    return nc


_CACHE = {}
LAST_RESULTS = None


def _to_bf16(a):
    """float32 ndarray -> bfloat16 (ml_dtypes if present, else bit-trunc)."""
    try:
        import ml_dtypes

        return a.astype(ml_dtypes.bfloat16)
    except ImportError:
        f = np.ascontiguousarray(a, dtype=np.float32)
        return (f.view(np.uint32) >> 16).astype(np.uint16)


def kernel(queries, keys, values, mask):
    global LAST_RESULTS
    if "nc" not in _CACHE:
        _CACHE["nc"] = build_program()
    nc = _CACHE["nc"]

    qt = _to_bf16(np.ascontiguousarray(np.asarray(queries).transpose(0, 2, 1)))
    kt = _to_bf16(np.ascontiguousarray(np.asarray(keys).transpose(0, 2, 1)))
    vb = _to_bf16(np.ascontiguousarray(np.asarray(values)))
    keep_u8 = np.ascontiguousarray(
        (~np.asarray(mask)).transpose(0, 2, 1)
    ).view(np.uint8).copy()
    # M2 chunks: PE adds -448*value pre-exp, so those key rows carry the
    # masked (not keep) polarity.
    for c in sorted(M2_CHUNKS):
        keep_u8[:, c * KCH : (c + 1) * KCH, :] ^= 1
    # pre-arrange into the SBUF slab layout so each SWDGE cast DMA reads
    # contiguous 4KB rows: [h, qt, qh, p, (qcl kl j)]
    # k = qt*512 + kl*128 + p ; q = (qh*2 + qcl)*512 + j
    keep_u8 = keep_u8.reshape(BH, 4, KLQ, P, 2, 2, QCH)
    keep_u8 = np.ascontiguousarray(keep_u8.transpose(0, 1, 4, 3, 5, 2, 6))
    keep_u8 = keep_u8.reshape(BH, 4, 2, P, 2 * KLQ * QCH)

    in_maps = []
    for cix in range(N_CORES):
        sl = slice(cix * H_PER_CORE, (cix + 1) * H_PER_CORE)
        in_maps.append(
            {
                "q": qt[sl],
                "k": kt[sl],
                "v": vb[sl],
                "m": keep_u8[sl],
            }
        )

    trace = bool(int(os.environ.get("ATTN_TRACE", "0")))
    res = run_bass_kernel_spmd(
        nc, in_maps, core_ids=list(range(N_CORES)), trace=trace
    )
    LAST_RESULTS = res
    # o2: [H, n_qc, D+1, QCH] unnormalized O^T; divide by the denominator
    # row and transpose back to [H, S, D] on the host
    o2 = np.concatenate([r["o"] for r in res.results], axis=0)
    on = o2[:, :, :D, :] / o2[:, :, D : D + 1, :]
    return np.ascontiguousarray(on.transpose(0, 1, 3, 2)).reshape(BH, S_FULL, D)
